# revision 53
# baseline (speedup 1.0000x reference)
"""Trainium2 Bass kernel for the CAM (channel attention module) problem.

Computation (per batch b):
    A = inputs[b] reshaped [N=4096, C=512]
    G = A^T A                       (channel Gram matrix, [C, C])
    attn = softmax(G, axis=-1)
    out[b] = gamma * (A @ attn^T) + A

Distribution: pure data-parallel over the batch dim: 16 batches over 8
NeuronCores = 2 batches/core. No collectives.

Design notes (v12), all constants HW-measured on this part:
  - HBM traffic 21MB/core: x bf16 (8.4) + host-pretransposed A^T fp8
    (4.2) + y bf16 (8.4).  The fp8 gram operand is cast on-chip (DVE).
  - All LOADS ride the sync (HWDGE) ring, which executes FIFO at
    ~350GB/s, ordered to match consumption: x_b0, x_b1 g0-g3 with the
    A^T_b0 chunks woven between them, then A^T_b1.  STORES ride the
    gpsimd (SWDGE) ring, which round-robins its whole queue
    concurrently -- fine for stores, disastrous for ordered loads.
  - Eras: gram_b0, gram_b1, mm2_b0, mm2_b1.  Each batch's softmax chain
    is emitted as closures interleaved into the NEXT era's PE stream so
    its ~8us serial engine-hop chain hides under matmuls.
  - softmax fused:  Tw[j,i] = exp(G[j,i] + V_i),  V = -m + ln(|g|/s),
    one rank-1 broadcast of V instead of separate -m and w broadcasts,
    and one tensor op + one exp per j-tile (gamma==0 gives V=-inf,
    exp->0, exactly right; gamma's SIGN is folded into xT8 on the
    host).  Row-blocks of the exp-sum pass start as soon as their
    lower-triangle rebuild lands (block 0 needs none).
  - mm2 residual (po + A) drain mix per measured costs:
      a: DVE tensor_tensor(po_psum + A)         ~690ns
      q: PE  po += I_bf16 @ A_tile (~110ns), then one ACT copy ~830ns
      u: ACT copy -> Sg, DVE bf16 add           ~440ns DVE
    po rotates over 7 PSUM banks (pPo 3 + the idle gram banks) so the
    MM->drain->MM semaphore round-trip (~2.2us) never stalls the PE.
  - Gs is bf16 (softmax has ~3500 margin to underflow; bf16's +-16 on
    G~4096 is free and the diagonal cancels exactly).
"""

import sys

if "/opt/trn_rl_repo" not in sys.path:
    sys.path.insert(0, "/opt/trn_rl_repo")

import numpy as np

B, H, W, C = 16, 64, 64, 512
N = H * W                 # 4096
NCORES = 8
BPC = B // NCORES         # batches per core = 2
P = 128                   # partitions
NT = N // P               # 32 n-tiles
CT = C // P               # 4 channel tiles
NGRP = 4                  # n-tile groups per batch
GNT = NT // NGRP          # 8 n-tiles per group
OG = 4                    # n-tiles per output store group

_BUILD_CACHE = {}


def _ml_bf16():
    import ml_dtypes

    return np.dtype(ml_dtypes.bfloat16)


def build_bass(gamma_val: float):
    import concourse.bass as bass
    import concourse.bacc as bacc
    import concourse.tile as tile
    from concourse import mybir
    from contextlib import ExitStack

    f32 = mybir.dt.float32
    bf16 = mybir.dt.bfloat16
    f8 = mybir.dt.float8e4
    DR = mybir.MatmulPerfMode.DoubleRow
    Exp = mybir.ActivationFunctionType.Exp
    Ln = mybir.ActivationFunctionType.Ln
    Alu = mybir.AluOpType
    AX = mybir.AxisListType

    abs_gamma = abs(float(gamma_val))

    nc = bacc.Bacc("TRN2", target_bir_lowering=False)
    x = nc.dram_tensor("x", [BPC, P, NT, C], bf16, kind="ExternalInput")
    xT8 = nc.dram_tensor("xT8", [BPC, P, CT, N], f8, kind="ExternalInput")
    ident = nc.dram_tensor("ident", [P, P], f32, kind="ExternalInput")
    ident_h = nc.dram_tensor("ident_h", [P, P], bf16, kind="ExternalInput")
    ones_f = nc.dram_tensor("ones_f", [1, P], f32, kind="ExternalInput")
    y = nc.dram_tensor("y", [BPC, P, NT, C], bf16, kind="ExternalOutput")

    with tile.TileContext(nc) as tc, ExitStack() as ctx:
        singles = ctx.enter_context(tc.tile_pool(name="singles", bufs=1))
        pA = ctx.enter_context(tc.tile_pool(name="pA", bufs=2))
        pA8 = ctx.enter_context(tc.tile_pool(name="pA8", bufs=2))
        pAT = ctx.enter_context(tc.tile_pool(name="pAT", bufs=2))
        pGs = ctx.enter_context(tc.tile_pool(name="pGs", bufs=2))
        pSm = ctx.enter_context(tc.tile_pool(name="pSm", bufs=2))
        pTmp = ctx.enter_context(tc.tile_pool(name="pTmp", bufs=2))
        pTw = ctx.enter_context(tc.tile_pool(name="pTw", bufs=2))
        pSg = ctx.enter_context(tc.tile_pool(name="pSg", bufs=3))
        pOut = ctx.enter_context(tc.tile_pool(name="pOut", bufs=10))
        pG = ctx.enter_context(tc.tile_pool(name="pG", bufs=4, space="PSUM"))
        pPv = ctx.enter_context(tc.tile_pool(name="pPv", bufs=1, space="PSUM"))
        pPo = ctx.enter_context(tc.tile_pool(name="pPo", bufs=3, space="PSUM"))

        sb_ident = singles.tile([P, P], f32)
        nc.gpsimd.dma_start(out=sb_ident, in_=ident[:, :])
        sb_ident_h = singles.tile([P, P], bf16)
        nc.gpsimd.dma_start(out=sb_ident_h, in_=ident_h[:, :])
        sb_ones_f = singles.tile([1, P], f32)
        nc.gpsimd.dma_start(out=sb_ones_f, in_=ones_f[:, :])

        st = [dict() for _ in range(BPC)]

        def emit_x(b, g_lo, g_hi):
            Ab = st[b]["A"]
            for g in range(g_lo, g_hi):
                if b == 0 and g == 0:
                    for h in range(GNT // 2):
                        nc.sync.dma_start(
                            out=Ab[:, 2 * h:2 * h + 2, :],
                            in_=x[b, :, 2 * h:2 * h + 2, :],
                        )
                elif b == 0:
                    # 4-nt chunks keep the cast->gram pipeline dense
                    for h in range(2):
                        sl = slice(g * GNT + 4 * h, g * GNT + 4 * h + 4)
                        nc.sync.dma_start(out=Ab[:, sl, :],
                                          in_=x[b, :, sl, :])
                else:
                    sl = slice(g * GNT, (g + 1) * GNT)
                    nc.sync.dma_start(out=Ab[:, sl, :], in_=x[b, :, sl, :])

        def emit_at(b, c_lo, c_hi):
            ATb = st[b]["AT"]
            NC4 = N // 4
            for c in range(c_lo, c_hi):
                sl = slice(c * NC4, (c + 1) * NC4)
                nc.sync.dma_start(out=ATb[:, :, sl], in_=xT8[b][:, :, sl])

        def emit_cast(b, fine_first=False):
            Ab, A8b = st[b]["A"], st[b]["A8"]
            chunks = []
            n0 = 0
            if fine_first:
                chunks += [(i * 2, (i + 1) * 2) for i in range(4)]
                n0 = GNT
            while n0 < NT:
                chunks.append((n0, n0 + 4))
                n0 += 4
            for lo, hi in chunks:
                nc.vector.tensor_copy(
                    out=A8b[:, lo:hi, :], in_=Ab[:, lo:hi, :]
                )

        # Gram (upper-triangle blocks), fp8 DoubleRow, with side closures
        def emit_gram(b, side_ops=()):
            side = list(side_ops)
            A8b = st[b]["A8"]
            G = [
                pG.tile([P, C], f32, name=f"G_b{b}c{ci}", tag="G")
                for ci in range(CT)
            ]
            NP2 = NT // 2
            for t in range(NP2):
                for ci in range(CT):
                    nc.tensor.matmul(
                        G[ci][:, ci * P:],
                        lhsT=A8b[:, 2 * t:2 * t + 2, ci * P:(ci + 1) * P],
                        rhs=A8b[:, 2 * t:2 * t + 2, ci * P:],
                        start=(t == 0),
                        stop=(t == NP2 - 1),
                        perf_mode=DR,
                    )
                if t >= 1:
                    for _ in range(2):
                        if side:
                            side.pop(0)()
            while side:
                side.pop(0)()
            st[b]["G"] = G

        # G rows PSUM->SBUF as bf16 (frees the G banks) + row-max (negated)
        def emit_stats(b):
            G = st[b]["G"]
            Gs = pGs.tile([P, CT, C], bf16, name=f"Gs_b{b}", tag="Gs")
            for ci in range(CT):
                eng = nc.vector.tensor_copy if ci % 2 == 0 else nc.scalar.copy
                eng(out=Gs[:, ci, ci * P:], in_=G[ci][:, ci * P:])
            negm = pSm.tile([P, CT], f32, name=f"negm_b{b}", tag="negm")
            for it in range(CT):
                nc.vector.tensor_reduce(
                    out=negm[:, it:it + 1],
                    in_=Gs[:, it, it * P:],
                    axis=AX.X,
                    op=Alu.max,
                    negate=True,
                )
            st[b]["Gs"] = Gs
            st[b]["negm"] = negm

        # softmax tail as closures for interleaving into the next PE era
        def softmax_closures(b):
            ve = nc.vector
            Gs = st[b]["Gs"]
            negm = st[b]["negm"]
            s_acc = pSm.tile([P, CT], f32, name=f"s_b{b}", tag="s")
            wrec = pSm.tile([P, CT], f32, name=f"w_b{b}", tag="w")
            vcol = pSm.tile([P, CT], f32, name=f"v_b{b}", tag="v")
            Tw8 = pTw.tile([P, CT, C], f8, name=f"Tw8_b{b}", tag="Tw")
            st[b]["Tw"] = Tw8
            ops = []

            QUADS = [[(1, 0), (2, 0), (2, 1), (3, 0)], [(3, 1), (3, 2)]]
            trq = [None, None]

            def blk_tq(qi, b=b, Gs=Gs):
                # trq sits in a pPo slot (idle during gram eras) so both
                # quad groups can be in flight at once
                trq[qi] = pPo.tile([P, C], bf16, name=f"trq_b{b}_{qi}",
                                   tag="po")
                for q, (it, jt) in enumerate(QUADS[qi]):
                    nc.tensor.transpose(
                        out=trq[qi][:, q * P:(q + 1) * P],
                        in_=Gs[:, jt, it * P:(it + 1) * P],
                        identity=sb_ident_h,
                    )

            def blk_cq(qi, b=b, Gs=Gs):
                for q, (it, jt) in enumerate(QUADS[qi]):
                    nc.scalar.copy(out=Gs[:, it, jt * P:(jt + 1) * P],
                                   in_=trq[qi][:, q * P:(q + 1) * P])

            def s_pass(its, b=b, Gs=Gs, negm=negm, s_acc=s_acc):
                for it in its:
                    S = pTmp.tile([P, C], bf16, name=f"S_b{b}t{it}", tag="S")
                    nc.scalar.activation(
                        out=S,
                        in_=Gs[:, it, :],
                        func=Exp,
                        bias=negm[:, it:it + 1],
                        scale=1.0,
                        accum_out=s_acc[:, it:it + 1],
                    )

            # V = -m + ln(|gamma|/s), one broadcast feeds the whole tail
            def v_col(b=b):
                nc.vector.reciprocal(out=wrec, in_=s_acc)
                lnw = pSm.tile([P, CT], f32, name=f"lnw_b{b}", tag="lnw")
                nc.scalar.activation(out=lnw, in_=wrec, func=Ln,
                                     scale=float(abs_gamma))
                ve.tensor_tensor(out=vcol, in0=negm, in1=lnw, op=Alu.add)
                # gamma==0 gives ln(0) = -inf; clamp so the PE transpose
                # (0 * -inf = NaN) stays finite.  exp(-30000) is still 0.
                ve.tensor_scalar_max(out=vcol, in0=vcol, scalar1=-30000.0)

            vrow = pSm.tile([1, C], f32, name=f"vrow_b{b}", tag="vrow")

            def c2r(b=b):
                vps = pPv.tile([1, C], f32, name=f"vps_b{b}", tag="pv")
                for it in range(CT):
                    nc.tensor.transpose(
                        out=vps[0:1, it * P:(it + 1) * P],
                        in_=vcol[:, it:it + 1],
                        identity=sb_ident,
                    )
                nc.scalar.copy(out=vrow, in_=vps)

            V_rep = pSm.tile([P, C], f32, name=f"vrep_b{b}", tag="vrep")

            def rank1(b=b):
                ps = pPv.tile([P, C], f32, name=f"vr_b{b}", tag="pv")
                nc.tensor.matmul(ps, lhsT=sb_ones_f, rhs=vrow,
                                 start=True, stop=True)
                nc.scalar.copy(out=V_rep, in_=ps)

            # T_w[j, i] = exp(G[j, i] + V_i)   (G symmetric)
            def tw_j(jt, b=b, Gs=Gs, Tw8=Tw8):
                tmp = pTmp.tile([P, C], bf16, name=f"tmp_b{b}j{jt}", tag="tmp")
                ve.tensor_tensor(
                    out=tmp, in0=Gs[:, jt, :], in1=V_rep, op=Alu.add
                )
                nc.scalar.activation(out=Tw8[:, jt, :], in_=tmp, func=Exp)

            ops.append(lambda: blk_tq(0))
            ops.append(lambda: (blk_cq(0), s_pass([0])))
            ops.append(lambda: (blk_tq(1), s_pass([1, 2])))
            ops.append(lambda: blk_cq(1))
            ops.append(lambda: (s_pass([3]), v_col()))
            ops.append(c2r)
            ops.append(rank1)
            for jt in range(CT):
                ops.append(lambda jt=jt: tw_j(jt))
            return ops

        # second matmul + residual (po + A, gamma folded into Tw) + store
        def emit_mm2(b, og_lo=0, og_hi=None, side_ops=()):
            side = list(side_ops)
            Ab = st[b]["A"]
            ATb = st[b]["AT"]
            Tw8 = st[b]["Tw"]
            NOG = NT // OG
            if og_hi is None:
                og_hi = NOG
            for og in range(og_lo, og_hi):
                outg = pOut.tile(
                    [P, OG, C], bf16, name=f"out_b{b}g{og}", tag="out"
                )
                for k in range(OG):
                    nt = og * OG + k
                    if b == 0:
                        # first two ogs drain DVE-only so their stores fire
                        # before the sm1 chain clears the ACT queue
                        kind = ("a" if og < 2 else
                                ("a", "q", "a",
                                 "q" if og % 2 == 0 else "a")[k])
                    else:
                        kind = ("a", "q", "a", "q")[k]
                    # po rotates over 7 banks: pPo's 3 + the idle gram 4
                    if nt % 7 < 3:
                        po = pPo.tile([P, C], f32, name=f"po_b{b}n{nt}",
                                      tag="po")
                    else:
                        po = pG.tile([P, C], f32, name=f"po_b{b}n{nt}",
                                     tag="G")
                    for u in range(CT // 2):
                        nc.tensor.matmul(
                            po,
                            lhsT=ATb[:, 2 * u:2 * u + 2, nt * P:(nt + 1) * P],
                            rhs=Tw8[:, 2 * u:2 * u + 2, :],
                            start=(u == 0),
                            stop=(u == CT // 2 - 1 and kind != "q"),
                            perf_mode=DR,
                        )
                    if kind == "q":
                        nc.tensor.matmul(
                            po,
                            lhsT=sb_ident_h,
                            rhs=Ab[:, nt, :],
                            start=False,
                            stop=True,
                        )
                        nc.scalar.copy(out=outg[:, k, :], in_=po)
                    elif kind == "a":
                        nc.vector.tensor_tensor(
                            out=outg[:, k, :], in0=po, in1=Ab[:, nt, :],
                            op=Alu.add,
                        )
                    else:
                        Sgt = pSg.tile([P, C], bf16,
                                       name=f"Sg_b{b}n{nt}", tag="Sg")
                        nc.scalar.copy(out=Sgt, in_=po)
                        nc.vector.tensor_tensor(
                            out=outg[:, k, :], in0=Sgt, in1=Ab[:, nt, :],
                            op=Alu.add,
                        )
                    if side:
                        side.pop(0)()
                # b0 stores ride gpsimd (overlapping the b1 loads on sync);
                # b1 stores ride sync, which is idle and strictly FIFO, so
                # the tail drains at full HWDGE rate.  The final store is
                # split across both rings to shorten the tail.
                dst = y[b, :, og * OG:(og + 1) * OG, :]
                if b == 1 and og == NOG - 1:
                    half = OG // 2
                    nc.gpsimd.dma_start(out=dst[:, :half, :],
                                        in_=outg[:, :half, :])
                    nc.sync.dma_start(out=dst[:, half:, :],
                                      in_=outg[:, half:, :])
                else:
                    eng = nc.gpsimd if b == 0 else nc.sync
                    eng.dma_start(out=dst, in_=outg)
            while side:
                side.pop(0)()

        # ---- PE warm-up --------------------------------------------------
        warm_sb = pSm.tile([P, P], bf16, name="warm_sb", tag="warmsb")
        nc.vector.memset(warm_sb, 0.0)
        warm_ps = pPo.tile([P, P], f32, name="warm_ps", tag="po")
        for _ in range(40):
            nc.tensor.matmul(warm_ps, lhsT=warm_sb, rhs=warm_sb,
                             start=True, stop=True)

        # ---- allocations -------------------------------------------------
        for b in range(BPC):
            st[b]["A"] = pA.tile([P, NT, C], bf16, name=f"A_b{b}", tag="A")
            st[b]["A8"] = pA8.tile([P, NT, C], f8, name=f"A8_b{b}", tag="A8")
            st[b]["AT"] = pAT.tile([P, CT, N], f8, name=f"AT_b{b}", tag="AT")

        # ---- sync-ring load order (arrival == consumption) ---------------
        # x_b1 fully ahead of A^T_b0: gram_b1's end (x_b1-paced) gates
        # mm2_b0's start, while A^T_b0's first chunks still land in time
        emit_x(0, 0, 4)        # x_b0
        emit_x(1, 0, 4)        # x_b1
        emit_at(0, 0, 4)       # A^T_b0
        emit_at(1, 0, 4)       # A^T_b1

        # ---- compute schedule -------------------------------------------
        emit_cast(0, fine_first=True)
        emit_gram(0)
        emit_stats(0)
        emit_cast(1)           # paced by the x_b1 loads
        emit_gram(1, side_ops=softmax_closures(0))
        emit_stats(1)
        # mm2_b0's first two ogs (all-DVE drains) go ahead of sm1 so their
        # stores fire early; then sm1's serial chain runs on clean queues
        # overlapping the rest of mm2_b0's matmuls, so Tw_b1 is ready when
        # mm2_b0's PE stream ends
        emit_mm2(0, 0, 2)
        for op in softmax_closures(1):
            op()
        emit_mm2(0, 2)
        emit_mm2(1)

    nc.compile()
    return nc


def run(inputs_arr: np.ndarray, gamma_val: float, trace: bool = False):
    """Compile + run on the 8 cores. Returns (output [16,64,64,512], results)."""
    from concourse.bass_utils import run_bass_kernel_spmd

    key = round(float(gamma_val), 12)
    if key not in _BUILD_CACHE:
        _BUILD_CACHE[key] = build_bass(float(gamma_val))
    nc = _BUILD_CACHE[key]

    import ml_dtypes

    bf16 = _ml_bf16()
    f8 = np.dtype(ml_dtypes.float8_e4m3)
    xs = np.asarray(inputs_arr, dtype=np.float32).reshape(B, N, C).astype(bf16)
    xs_t = np.ascontiguousarray(
        xs.reshape(B, NT, P, C).transpose(0, 2, 1, 3)
    )
    # gamma's sign rides on xT8 (the kernel uses |gamma| in the softmax)
    sgn = -1.0 if gamma_val < 0 else 1.0
    xsT8 = (
        (xs.astype(np.float32) * sgn).astype(f8).transpose(0, 2, 1)
        .reshape(B, CT, P, N).transpose(0, 2, 1, 3)
    )
    xsT8 = np.ascontiguousarray(xsT8)
    eye = np.eye(P, dtype=np.float32)
    eye_h = eye.astype(bf16)
    ones_f = np.ones((1, P), dtype=np.float32)
    in_maps = [
        {
            "x": xs_t[c * BPC:(c + 1) * BPC],
            "xT8": xsT8[c * BPC:(c + 1) * BPC],
            "ident": eye,
            "ident_h": eye_h,
            "ones_f": ones_f,
        }
        for c in range(NCORES)
    ]
    res = run_bass_kernel_spmd(nc, in_maps, list(range(NCORES)), trace=trace)
    out = np.concatenate(
        [np.asarray(res.results[c]["y"]) for c in range(NCORES)], axis=0
    )
    out = out.transpose(0, 2, 1, 3).reshape(B, N, C)
    return out.astype(np.float32).reshape(B, H, W, C), res


def kernel(inputs: np.ndarray, gamma: np.ndarray) -> np.ndarray:
    gamma_val = float(np.asarray(gamma).reshape(-1)[0])
    out, _ = run(inputs, gamma_val, trace=False)
    return out.astype(np.float32)


if __name__ == "__main__":
    rng = np.random.default_rng(0)
    inp = rng.standard_normal((B, H, W, C), dtype=np.float32)
    gam = np.zeros((1,), dtype=np.float32)
    out = kernel(inp, gam)
    print("shape", out.shape, "dtype", out.dtype)
    print("max|out - inp| =", np.abs(out - inp).max())
